# revision 5
# baseline (speedup 1.0000x reference)
"""DeepPoly ReLU transformer back-substitution on 8 trn2 NeuronCores.

Math (reference, per output row n of weight W [N, M]):
    l, u = bounds;  ind2 = l>=0;  ind3 = (u>0)&(l<0)
    beta = 1[ind2];  lmbda = ind2?1 : ind3? u/(u-l) : 0;  mu = ind3? -l*u/(u-l) : 0
    a := Wp@in_l + Wn@in_u;  b := Wp@in_u + Wn@in_l   (Wp/Wn = pos/neg split)
    new_l = beta*(a+bias);  new_u = lmbda*(b+bias)+mu
    lb = max(ind2? l:0, new_l);  ub = min(ind2|ind3? u:0, new_u)

With s := W@(in_l+in_u) and t := |W|@(in_l-in_u), the splits collapse to
    a = (s+t)/2,  b = (s-t)/2
so the device only needs TWO matvec streams: W against (in_l+in_u) and
|W| against (in_l-in_u).

The device streams the 8 MB fp8(e4m3) W^T shard (row-shard of N/8=1024
output rows per core, host-scaled by 64) through the PE as the moving
operand in DoubleRow perf mode (two 128-row k-subtiles per matmul, 0.5
cycles/output column).  |W| is produced on the DVE as a packed sign-bit
clear: the fp8 tile is bitcast to uint16 and ANDed with 0x7f7f, so each
DVE lane strips two sign bits per element at 16-bit 4x throughput.  The
W pass and the |W| pass run concurrently as column-tiled matmuls
(tile_position col groups 0 and 32).  PSUM accumulates fp32 over the
M=8192 contraction; results are DVE-copied to SBUF and DMA'd out raw.
All O(N)/O(M) prep (coefficients, W transpose/tiling/scale/fp8 cast)
and the O(N) epilogue (bias add, beta/lmbda scaling, clamping) run on
host in fp32.
"""

import numpy as np

import concourse.bass as bass
import concourse.mybir as mybir
from concourse.tile import TileContext
from concourse.bass_utils import run_bass_kernel_spmd

N = 8192          # output rows of W
M = 8192          # contraction dim (input features)
NC = 8            # cores
NPC = N // NC     # 1024 output rows per core
MT = M // 128     # 64 contraction subtiles of 128
NPAIR = MT // 2   # 32 DoubleRow subtile pairs
VK = 16           # stationary entries per m-subtile (DoubleRow needs pair
                  # stride %16==0 and an even column count per load)
NCHUNK = NPC // 512  # 2 PSUM chunks of 512 columns

F32 = mybir.dt.float32
FP8 = mybir.dt.float8e4
U16 = mybir.dt.uint16
WSCALE = 64.0     # host premultiplies W so fp8(e4m3) keeps full mantissa

# DMA tile schedule, in 128-row m-subtiles per transfer (128 KB each in
# fp8).  Small leading tiles cut the latency to the first matmul; 2 MiB
# steady-state tiles maximize per-descriptor packet size (HBM
# efficiency); the taper at the end keeps the post-stream compute tail
# short.  Entries must be even (DoubleRow pairs) and sum to MT.
TILE_SCHED = [2, 2, 4, 8, 16, 16, 8, 4, 2, 2]
assert sum(TILE_SCHED) == MT
assert all(A % 2 == 0 for A in TILE_SCHED)

N_WARM = 12  # cold-clock PE warmup matmuls issued during the DMA preamble

_nc_cache = {}


def _build(mm_dt):
    nc = bass.Bass()
    assert mm_dt == FP8, "kernel tuned for fp8e4 DoubleRow matmuls"
    # host pre-tiles W^T so each [128, A*NPC] DMA tile is one contiguous
    # block: tile t partition p holds rows {m0_t + a*128 + p} of W^T[:, core]
    wt = nc.dram_tensor("wt", [M * NPC], mm_dt, kind="ExternalInput")
    # per m-subtile stationary block: [su, dd, dd, 0, pad...] (su=in_l+in_u,
    # dd=in_l-in_u); col group 0 loads cols 0:2, col group 1 loads cols 2:4
    vecs = nc.dram_tensor("vecs", [128, VK * MT], mm_dt, kind="ExternalInput")
    outm = nc.dram_tensor("outm", [2, NPC], F32, kind="ExternalOutput")

    with TileContext(nc) as tc:
        with (
            tc.tile_pool(name="wpool", bufs=3) as wpool,
            tc.tile_pool(name="wppool", bufs=3) as wppool,
            tc.tile_pool(name="const", bufs=1) as cpool,
            tc.tile_pool(name="psum", bufs=1, space="PSUM") as ppool,
            tc.tile_pool(name="epil", bufs=1) as epool,
        ):
            vecs_sb = cpool.tile([128, VK * MT], mm_dt, tag="vecs")
            nc.scalar.dma_start(out=vecs_sb[:], in_=vecs[:])
            vecs3 = vecs_sb[:].rearrange("p (m k) -> p m k", k=VK)

            # PE warmup: dep-free matmuls on memset scratch keep the PE busy
            # through the HAM SHORT window while the first W tile loads, so
            # real matmuls run at 2.4 GHz from the start.
            scratch = cpool.tile([128, 512], mm_dt, tag="scratch")
            nc.gpsimd.memset(scratch[:], 0.0)
            warm_ps = ppool.tile([2, 512], F32, tag="warm", name="warm_ps")
            for _ in range(N_WARM):
                nc.tensor.matmul(
                    warm_ps[:],
                    scratch[:, 0:2],
                    scratch[:, 0:512],
                    start=True,
                    stop=True,
                )

            # DoubleRow outputs must sit at PSUM partition base 0 (any
            # nonzero PE column position fails the ISA check), so the W and
            # |W| streams accumulate into separate PSUM banks.
            # rows: ps_s = [s = W@su, W@dd(ignored)], ps_t = [t = |W|@dd, 0]
            ps_s = [
                ppool.tile([2, 512], F32, tag=f"pss{c}", name=f"pss{c}")
                for c in range(NCHUNK)
            ]
            ps_t = [
                ppool.tile([2, 512], F32, tag=f"pst{c}", name=f"pst{c}")
                for c in range(NCHUNK)
            ]

            mt = 0
            ofs = 0
            for ti, A in enumerate(TILE_SCHED):
                w = wpool.tile([128, A * NPC], mm_dt, tag="w", name="w")
                # alternate between the two HWDGE rings (SP / ACT) so W-tile
                # transfers pipeline instead of serializing on one ring
                dma_eng = nc.sync if ti % 2 == 0 else nc.scalar
                dma_eng.dma_start(
                    out=w[:],
                    in_=wt[ofs : ofs + 128 * A * NPC].rearrange(
                        "(p f) -> p f", p=128
                    ),
                )
                ofs += 128 * A * NPC
                wp = wppool.tile([128, A * NPC], mm_dt, tag="wp", name="wp")
                # |W|: clear fp8 sign bits, two lanes per 16-bit DVE element
                nc.vector.tensor_scalar(
                    out=wp[:].bitcast(U16),
                    in0=w[:].bitcast(U16),
                    scalar1=0x7F7F,
                    scalar2=None,
                    op0=mybir.AluOpType.bitwise_and,
                )
                w3 = w[:].rearrange("p (a n) -> p a n", a=A)
                wp3 = wp[:].rearrange("p (a n) -> p a n", a=A)
                for a in range(0, A, 2):
                    pr = mt // 2
                    for c in range(NCHUNK):
                        lo = c * 512
                        nc.tensor.matmul(
                            ps_s[c][0:2, :],
                            vecs3[:, mt : mt + 2, 0:2],
                            w3[:, a : a + 2, lo : lo + 512],
                            start=(pr == 0),
                            stop=(pr == NPAIR - 1),
                            perf_mode=mybir.MatmulPerfMode.DoubleRow,
                            skip_group_check=True,
                        )
                        nc.tensor.matmul(
                            ps_t[c][0:2, :],
                            vecs3[:, mt : mt + 2, 2:4],
                            wp3[:, a : a + 2, lo : lo + 512],
                            start=(pr == 0),
                            stop=(pr == NPAIR - 1),
                            perf_mode=mybir.MatmulPerfMode.DoubleRow,
                            skip_group_check=True,
                        )
                    mt += 2
                # dep-free filler matmuls at every tile boundary: in the
                # DMA-bound steady state the PE stalls per tile, and
                # clustered stalls cross the ~3.4us HAM window, re-throttling
                # the PE to 1.2 GHz.  The fillers run inside each gap (the PE
                # queue is in-order) and break up the idle stretches so real
                # matmuls stay at 2.4 GHz.
                if ti < len(TILE_SCHED) - 1:
                    for _ in range(3 if ti < 3 else 2):
                        nc.tensor.matmul(
                            warm_ps[:],
                            scratch[:, 0:2],
                            scratch[:, 0:512],
                            start=True,
                            stop=True,
                        )

            # evacuate PSUM on the DVE only (scalar.copy would pay a 1.3us
            # ACT_TABLE_LOAD) and DMA the result out; SBUF partition offsets
            # must be quarter-aligned, so s lands on row 0 and t on row 32
            om_sb = epool.tile([33, NPC], F32, tag="om")
            for c in range(NCHUNK):
                sl = slice(c * 512, (c + 1) * 512)
                nc.vector.tensor_copy(om_sb[0:1, sl], ps_s[c][0:1, :])
                nc.vector.tensor_copy(om_sb[32:33, sl], ps_t[c][0:1, :])
            nc.sync.dma_start(out=outm[0:1, :], in_=om_sb[0:1, :])
            nc.scalar.dma_start(out=outm[1:2, :], in_=om_sb[32:33, :])
    return nc


def _legalize_sync_waits(nc):
    """The walrus codegen in this toolchain accepts at most ONE sync-wait per
    instruction ("Too many sync wait commands").  Tile freely attaches
    several.  Hoist all but the last wait of each offending instruction onto
    same-engine NOPs spliced immediately before it — same-queue waits execute
    in order, so semantics are identical."""
    nop_map = {}
    all_nops = set()
    for f in nc.m.functions:
        for b in f.blocks:
            for inst in list(b.instructions):
                si = inst.sync_info
                if not (si and si.on_wait and len(si.on_wait) > 1):
                    continue
                waits = list(si.on_wait)
                nops = []
                for w in waits[:-1]:
                    # engine.nop() appends to the current (last) bb; the
                    # splice below removes it from wherever it landed and
                    # re-inserts it right before its target instruction.
                    nop = nc.engines[inst.engine].nop()
                    nop.ins.sync_info = mybir.SyncInfo(on_wait=[w], on_update=[])
                    nops.append(nop.ins)
                    all_nops.add(nop.ins.name)
                inst.sync_info = mybir.SyncInfo(
                    on_wait=[waits[-1]], on_update=list(si.on_update or [])
                )
                nop_map[inst.name] = nops
    if not nop_map:
        return
    for f in nc.m.functions:
        for b in f.blocks:
            insts = b.instructions
            new_list = []
            for inst in insts:
                if inst.name in all_nops:
                    continue
                for nop in nop_map.get(inst.name, ()):
                    new_list.append(nop)
                new_list.append(inst)
            insts[:] = new_list


def get_nc(mm_dt=FP8):
    key = str(mm_dt)
    if key not in _nc_cache:
        nc = _build(mm_dt)
        _legalize_sync_waits(nc)
        _nc_cache[key] = nc
    return _nc_cache[key]


def host_prep(bounds, weight, bias, in_lower, in_upper, mm_np=None):
    import ml_dtypes

    if mm_np is None:
        mm_np = ml_dtypes.float8_e4m3
    f32 = np.float32
    weight = np.asarray(weight, f32)
    in_lower = np.asarray(in_lower, f32)
    in_upper = np.asarray(in_upper, f32)

    su = (in_lower + in_upper).astype(f32)
    dd = (in_lower - in_upper).astype(f32)
    # per m-subtile stationary block: [su, dd, dd, 0, pad...]
    mvecs = np.zeros((M, VK), f32)
    mvecs[:, 0] = su
    mvecs[:, 1] = dd
    mvecs[:, 2] = dd
    mvecs = mvecs.astype(mm_np)
    vecs = np.ascontiguousarray(
        mvecs.reshape(MT, 128, VK).transpose(1, 0, 2).reshape(128, VK * MT)
    )

    WT = np.ascontiguousarray((weight.T * f32(WSCALE)).astype(mm_np))  # [M, N]
    in_maps = []
    for c in range(NC):
        sl = slice(c * NPC, (c + 1) * NPC)
        Wc = WT[:, sl]
        blocks = []
        m0 = 0
        for A in TILE_SCHED:
            blocks.append(
                Wc[m0 : m0 + A * 128]
                .reshape(A, 128, NPC)
                .transpose(1, 0, 2)
                .reshape(-1)
            )
            m0 += A * 128
        wt_flat = np.ascontiguousarray(np.concatenate(blocks))
        in_maps.append({"wt": wt_flat, "vecs": vecs})
    return in_maps


def assemble(results, bounds, bias):
    """Host epilogue: combine the raw matvecs with the O(N) DeepPoly
    coefficient math, exactly mirroring the reference formulas in fp32."""
    f32 = np.float32
    bounds = np.asarray(bounds, f32)
    bias = np.asarray(bias, f32)
    l, u = bounds[0], bounds[1]
    ind2 = l >= 0
    ind3 = (u > 0) & (l < 0)
    one, zero = f32(1.0), f32(0.0)
    diff = np.where(ind3, u - l, one).astype(f32)
    lmbda = np.where(ind2, one, np.where(ind3, u / diff, zero)).astype(f32)
    beta = np.where(ind2, one, zero).astype(f32)
    mu = np.where(ind3, -l * u / diff, zero).astype(f32)
    lb0 = np.where(ind2, l, zero).astype(f32)
    ub0 = np.where(ind2, u, np.where(ind3, u, zero)).astype(f32)

    s = np.empty(N, f32)
    t = np.empty(N, f32)
    for c, r in enumerate(results):
        sl = slice(c * NPC, (c + 1) * NPC)
        om = np.asarray(r["outm"])
        s[sl] = om[0]
        t[sl] = om[1]
    s /= f32(WSCALE)
    t /= f32(WSCALE)

    a = ((s + t) * f32(0.5)).astype(f32)  # Wp@in_l + Wn@in_u
    b = ((s - t) * f32(0.5)).astype(f32)  # Wp@in_u + Wn@in_l
    new_l = (beta * (a + bias)).astype(f32)
    new_u = (lmbda * (b + bias) + mu).astype(f32)
    lb = np.maximum(lb0, new_l)
    ub = np.minimum(ub0, new_u)
    return np.stack([lb, ub]).astype(f32)


def kernel(bounds, weight, bias, in_lower, in_upper):
    nc = get_nc()
    in_maps = host_prep(bounds, weight, bias, in_lower, in_upper)
    res = run_bass_kernel_spmd(nc, in_maps, list(range(NC)))
    return assemble(res.results, bounds, bias)
